# revision 20
# baseline (speedup 1.0000x reference)
"""Trainium2 Bass kernel for nn_Head (single-head causal attention).

Contract: kernel(**inputs) takes FULL inputs (x [8,2048,1024] f32,
Wk/Wq/Wv [64,1024] f32) and returns the FULL output [8,2048,64] f32.
Data-parallel over batch B=8 across the 8 NeuronCores (one batch row per
core); each core runs an identical single-core program.

v6 design (strip-pipelined, PSUM-direct exp, proj interleaved):
  - Host packs inputs in SBUF layout so every DMA moves 4-8KB contiguous
    lines per partition (DMA descriptor issue cost is per line: 1KB-line
    patterns took ~2.7us per strip to issue, 8KB lines ~0.6us):
      xh  [4, 128, 8, 512]  xT strips (strip-major)
      wh  [128, 2, 8, 128]  w[0]=[Wv;Wq], w[1]=[Wk/32;Wk/32]
      ch  [128, 1024] u8    constants (identities, tri mask)
    All input DMAs ride ONE ring (sync) so transfers complete strictly in
    strip order at full bandwidth.
  - Projections per strip: P1=[Wv;Wq] -> vT @ partitions 0:64 + qT @
    64:128; P2=[Wk/32;Wk/32] -> kT(scaled) @ 64:128 (kk2).  Everything
    the attention loop needs then lives at partitions 64:128 (q weights,
    kT rhs) with NO cross-partition moves (DVE/ACT/GpSimd are lane-wise;
    only PE transposes or DMA can cross, and both proved expensive).
  - ST: s-tile i as a K=64 matmul on the upper PE half (q_i stationary,
    kk2 streaming).  Two consecutive tiles land in one [128,2,512]
    two-bank PSUM group -> ONE exp per group DIRECTLY from PSUM (ACT
    reads PSUM at full rate, DVE at ~half rate) -> tri-mask diagonal
    blocks on DVE (bf16) -> PV row-tiled pairs: vaug/pt upper and lower
    halves run in LOCKSTEP (same tile, same columns) into OT_A/OT_B;
    the vaug ones-column accumulates the softmax denominator.
  - Schedule: the ST/exp phase of strip j is ACT-paced; the projections
    of strip j+1 and the PV of strip j-1 are interleaved BETWEEN ST
    groups so the PE never idles (idle fragments re-throttle the PE to
    1.2 GHz via HAM; it only re-warms after ~3.4us of sustained work).
  - Epilogue per strip: OT_A+OT_B -> SBUF, 4 PE transposes into one
    PSUM bank, reciprocal of the denominator + normalize on DVE,
    per-strip DMA out (issued from gpsimd).
  - Dummy matmuls cover the input-DMA latency so real work starts with
    the PE already warm (their PSUM is read once by a DVE copy so DCE
    keeps them).  A primer activation pulls the ~1.3us exp-table load
    into the DMA shadow.
"""

import sys

if "/opt/trn_rl_repo" not in sys.path:
    sys.path.insert(0, "/opt/trn_rl_repo")

import numpy as np

B = 8
T = 2048
C = 1024
H = 64
P = 128
CB = C // P        # 8 contraction chunks of 128
TJ = T // 512      # 4 column strips of 512
NT = T // P        # 16 s-tiles
N_CORES = 8

PIECES = [4 * j + 4 for j in range(TJ)]   # ST pieces per strip: 4, 8, 12, 16
N_DUMMY = 4                               # PE warm-up matmuls (N=512)

_NC_CACHE = {}


def _build_nc():
    import concourse.bass as bass
    import concourse.mybir as mybir
    import concourse.tile as tile
    from concourse.bass import ts

    fp32 = mybir.dt.float32
    bf16 = mybir.dt.bfloat16
    u8 = mybir.dt.uint8
    EXP = mybir.ActivationFunctionType.Exp
    MULT = mybir.AluOpType.mult
    ADD = mybir.AluOpType.add

    nc = bass.Bass(target_bir_lowering=False, debug=False)
    xh_d = nc.declare_dram_parameter("xh", [TJ, P, CB, 512], bf16, isOutput=False)
    wh_d = nc.declare_dram_parameter("wh", [P, 2, CB, P], bf16, isOutput=False)
    ch_d = nc.declare_dram_parameter("ch", [P, 1024], u8, isOutput=False)
    out_d = nc.declare_dram_parameter("out", [T, H], fp32, isOutput=True)

    from contextlib import ExitStack

    with tile.TileContext(nc) as tc, ExitStack() as stk:
        pers = stk.enter_context(tc.tile_pool(name="pers", bufs=1))
        xt0p = [pers.tile([P, 2, 512], bf16, tag=f"xt0p{k}", name=f"xt0p{k}")
                for k in range(4)]
        xts = [None] + [
            pers.tile([P, CB, 512], bf16, tag=f"xt{j}", name=f"xt{j}")
            for j in range(1, TJ)
        ]
        w_sb = pers.tile([P, 2, CB, P], bf16, tag="w_sb", name="w_sb")
        c_sb = pers.tile([P, 1024], u8, tag="c_sb", name="c_sb")
        # vq: vT @ 0:64 (for v transposes) + qT @ 64:128 (ST weights)
        vq = [pers.tile([P, 512], bf16, tag=f"vq{j}", name=f"vq{j}") for j in range(TJ)]
        # kk2: kT(scaled) @ 64:128 (ST rhs); top half unused
        kk2 = [pers.tile([P, 512], bf16, tag=f"kk2{j}", name=f"kk2{j}") for j in range(TJ)]
        # per-GROUP pt tiles: dependency granularity (a PV chunk must only
        # wait for its own exp group, not the whole strip)
        pt_sb = [
            [pers.tile([P, 2, 512], bf16, tag=f"pt{j}_{g}", name=f"pt{j}_{g}")
             for g in range(PIECES[j] // 2)]
            for j in range(TJ)
        ]
        vaug = [pers.tile([P, 4, H + 1], bf16, tag=f"va{j}", name=f"va{j}") for j in range(TJ)]
        oadd = [pers.tile([H + 1, 512], fp32, tag=f"oa{j}", name=f"oa{j}") for j in range(TJ)]
        o_sb = [pers.tile([P, 4, H], fp32, tag=f"o{j}", name=f"o{j}") for j in range(TJ)]
        rec = [pers.tile([P, 4], fp32, tag=f"rc{j}", name=f"rc{j}") for j in range(TJ)]
        scr_w = pers.tile([P, P], bf16, tag="scr_w", name="scr_w")
        scr_x = pers.tile([P, 512], bf16, tag="scr_x", name="scr_x")
        scr_rd = pers.tile([P, 1], fp32, tag="scr_rd", name="scr_rd")
        prim = pers.tile([P, 1], fp32, tag="prim", name="prim")
        prim_o = pers.tile([P, 1], fp32, tag="prim_o", name="prim_o")

        # constant views (shipped via DMA in ch)
        identb_lo = c_sb[0:H, 0:128].bitcast(bf16)        # [64, 64] @ 0:64
        identb128 = c_sb[:, 128:384].bitcast(bf16)        # [128, 128] (unused)
        tri = c_sb[:, 384:640].bitcast(bf16)              # [128, 128]
        ident65 = c_sb[0 : H + 1, 640:900].bitcast(fp32)  # [65, 65]

        # ---- early gpsimd work first: scratch memsets (unblock dummies) ----
        nc.gpsimd.memset(scr_w[:], 0.0)
        nc.gpsimd.memset(scr_x[:], 0.0)
        nc.gpsimd.memset(prim[:], 0.0)
        for j in range(TJ):
            nc.gpsimd.memset(vaug[j][:, :, H], 1.0)

        # ---- input DMAs on 2 rings; strip 0 in 4 chunk-pair pieces so
        # the kq projection starts as soon as ~0.6MB has landed ----
        nc.sync.dma_start(xt0p[0][:], xh_d[0, :, 0:2, :])
        nc.scalar.dma_start(w_sb[:], wh_d[:])
        nc.sync.dma_start(xt0p[2][:], xh_d[0, :, 4:6, :])
        nc.scalar.dma_start(xt0p[1][:], xh_d[0, :, 2:4, :])
        nc.sync.dma_start(xt0p[3][:], xh_d[0, :, 6:8, :])
        nc.scalar.dma_start(c_sb[:], ch_d[:])
        nc.sync.dma_start(xts[1][:], xh_d[1])
        nc.scalar.dma_start(xts[2][:], xh_d[2])
        nc.sync.dma_start(xts[3][:], xh_d[3])

        # ---- scalar primer: pull the exp table load into the DMA shadow ----
        nc.scalar.activation(prim_o[:], prim[:], EXP)

        def xsrc(j, cb):
            if j == 0:
                return xt0p[cb // 2][:, cb % 2, :]
            return xts[j][:, cb, :]

        # ---- PE warm-up dummies (read once by DVE so DCE keeps them) ----
        with tc.tile_pool(name="scrp", bufs=1, space="PSUM") as scrp:
            scr_ps = scrp.tile([P, 512], fp32, tag="scr", name="scr_ps")
            for k in range(N_DUMMY):
                nc.tensor.matmul(
                    scr_ps, scr_w[:], scr_x[:], start=(k == 0), stop=(k == N_DUMMY - 1)
                )
            nc.vector.tensor_copy(scr_rd[:], scr_ps[:, 0:1])

        with (
            tc.tile_pool(name="prjp", bufs=1, space="PSUM") as prjp,   # 1 bank
            tc.tile_pool(name="tpp", bufs=1, space="PSUM") as tpp,     # 1 bank
            tc.tile_pool(name="stp", bufs=2, space="PSUM") as stp,     # 4 banks
            tc.tile_pool(name="otp", bufs=1, space="PSUM") as otp,     # 2 banks
        ):
            ot_a = otp.tile([H + 1, 512], fp32, tag="ota", name="ot_a")
            ot_b = otp.tile([H + 1, 512], fp32, tag="otb", name="ot_b")

            CB_ORDER = [0, 1, 4, 5, 2, 3, 6, 7]  # strip-0 DMA arrival order

            def proj_steps(j):
                """Generator: one projection matmul per step (17 steps:
                8 P1 chunks, drain, 8 P2 chunks, drain, v transposes)."""
                order = CB_ORDER if j == 0 else list(range(CB))
                p1 = prjp.tile([P, 512], fp32, tag="prj", name=f"p1_{j}")
                for ci, cb in enumerate(order):
                    nc.tensor.matmul(
                        p1, w_sb[:, 0, cb, :], xsrc(j, cb),
                        start=(ci == 0), stop=(ci == CB - 1),
                    )
                    yield
                nc.vector.tensor_copy(vq[j][:], p1)
                p2 = prjp.tile([P, 512], fp32, tag="prj", name=f"p2_{j}")
                for ci, cb in enumerate(order):
                    nc.tensor.matmul(
                        p2, w_sb[:, 1, cb, :], xsrc(j, cb),
                        start=(ci == 0), stop=(ci == CB - 1),
                    )
                    yield
                nc.vector.tensor_copy(kk2[j][H:P, :], p2[H:P, :])
                # v transposes: vq[j][0:64, block m] -> vaug[j][:, m, 0:64]
                vt = tpp.tile([P, 4, H], bf16, tag="tp", name=f"vt{j}")
                for m in range(4):
                    nc.tensor.transpose(vt[:, m, :], vq[j][0:H, ts(m, P)], identb_lo)
                    yield
                nc.vector.tensor_copy(vaug[j][:, :, 0:H], vt)

            def st_steps(j):
                """Generator: one ST group (two s-tiles + exp) per step."""
                for m in range(2 * j + 2):
                    iA, iB = 2 * m, 2 * m + 1
                    o = max(0, P * iA - 512 * j)
                    oB = max(0, P * iB - 512 * j)
                    s2 = stp.tile([P, 2, 512], fp32, tag="st", name=f"s{j}_{m}")
                    nc.tensor.matmul(
                        s2[:, 0, o:512],
                        vq[iA // 4][H:P, ts(iA % 4, P)],
                        kk2[j][H:P, o:512],
                        start=True, stop=True,
                    )
                    nc.tensor.matmul(
                        s2[:, 1, oB:512],
                        vq[iB // 4][H:P, ts(iB % 4, P)],
                        kk2[j][H:P, oB:512],
                        start=True, stop=True,
                    )
                    nc.scalar.activation(
                        pt_sb[j][m][:, :, o:512],
                        s2[:, :, o:512],
                        EXP,
                    )
                    # tri-mask diagonal pieces of this group right away
                    for k, i in ((0, iA), (1, iB)):
                        if 4 * j <= i <= 4 * j + 3:
                            od = P * i - 512 * j
                            nc.vector.tensor_tensor(
                                pt_sb[j][m][:, k, od : od + P],
                                pt_sb[j][m][:, k, od : od + P],
                                tri, MULT,
                            )
                    yield m

            def pv_steps(j):
                """Generator: one PV chunk (row-tiled lockstep pair) per step."""
                n = PIECES[j]
                for i in range(n):
                    o = max(0, P * i - 512 * j)
                    pg = pt_sb[j][i // 2]
                    nc.tensor.matmul(
                        ot_a[:, o:512],
                        vaug[i // 4][0:H, i % 4, :],
                        pg[0:H, i % 2, o:512],
                        start=(i == 0), stop=(i == n - 1),
                    )
                    nc.tensor.matmul(
                        ot_b[:, o:512],
                        vaug[i // 4][H:P, i % 4, :],
                        pg[H:P, i % 2, o:512],
                        start=(i == 0), stop=(i == n - 1),
                    )
                    yield

            def emit_epilogue(j):
                nc.scalar.copy(oadd[j][:], ot_a[:, :])
                nc.vector.tensor_tensor(oadd[j][:], ot_b[:, :], oadd[j][:], ADD)
                orr = tpp.tile([P, 4, H + 1], fp32, tag="tp", name=f"or{j}")
                for m in range(4):
                    nc.tensor.transpose(orr[:, m, :], oadd[j][:, ts(m, P)], ident65)
                nc.vector.reciprocal(rec[j][:, :], orr[:, :, H])
                for m in range(4):
                    nc.vector.tensor_scalar_mul(
                        o_sb[j][:, m, :], orr[:, m, 0:H], rec[j][:, m : m + 1]
                    )
                nc.gpsimd.dma_start(
                    out_d[ts(j, 512), :].rearrange("(m p) d -> p m d", p=P),
                    o_sb[j][:],
                )

            def drain(gen, k=1):
                """Advance a generator k steps; False when exhausted."""
                for _ in range(k):
                    if gen is None:
                        return None
                    if next(gen, "END") == "END":
                        return None
                return gen

            # ---- pipeline ----
            # strip 0 projections stand alone (nothing to interleave with)
            g = proj_steps(0)
            while drain(g) is not None:
                pass
            prj = None
            for j in range(TJ):
                # finish any leftover projection steps of THIS strip first
                while prj is not None:
                    prj = drain(prj)
                prj = proj_steps(j + 1) if j + 1 < TJ else None
                pv = pv_steps(j - 1) if j > 0 else None
                npairs = 2 * j + 2
                npv = PIECES[j - 1] if j > 0 else 0
                pvd = 0
                for m in st_steps(j):
                    pvt = (npv * (m + 1)) // npairs
                    pv = drain(pv, pvt - pvd)
                    pvd = pvt
                    # at most 3 projection steps per ST group; leftovers
                    # carry into the next iteration's head
                    prj = drain(prj, 3)
                while pv is not None:
                    pv = drain(pv)
                if j > 0:
                    emit_epilogue(j - 1)
            # tail: last strip's PV with a per-128-block epilogue interleaved
            j = TJ - 1
            n = PIECES[j]
            pv = pv_steps(j)
            for i in range(n):
                drain(pv)
                # OT block m is final once chunks with o <= 128*m are done:
                # chunk i is the last writer of block m iff i == 12 + m
                m = i - 12
                if 0 <= m < 4:
                    nc.vector.tensor_copy(oadd[j][:, ts(m, P)], ot_a[:, ts(m, P)])
                    nc.vector.tensor_tensor(
                        oadd[j][:, ts(m, P)], ot_b[:, ts(m, P)],
                        oadd[j][:, ts(m, P)], ADD,
                    )
                    orr = tpp.tile([P, H + 1], fp32, tag="tp", name=f"ox{m}")
                    nc.tensor.transpose(orr, oadd[j][:, ts(m, P)], ident65)
                    nc.vector.reciprocal(rec[j][:, m : m + 1], orr[:, H : H + 1])
                    nc.vector.tensor_scalar_mul(
                        o_sb[j][:, m, :], orr[:, 0:H], rec[j][:, m : m + 1]
                    )
            nc.gpsimd.dma_start(
                out_d[ts(j, 512), :].rearrange("(m p) d -> p m d", p=P),
                o_sb[j][:],
            )

    return nc


def _split_multiwaits(nc):
    """Walrus codegen only supports one sync-wait command per instruction;
    hoist extra waits onto NoOps inserted just before (same engine queue,
    identical semantics since engines execute their queue in order)."""
    import concourse.mybir as mybir

    n = 0
    for fn in nc.m.functions:
        for block in fn.blocks:
            new_insts = []
            for inst in block.instructions:
                si = inst.sync_info
                if si is not None and si.on_wait and len(si.on_wait) > 1:
                    waits = list(si.on_wait)
                    for w in waits[:-1]:
                        n += 1
                        new_insts.append(
                            mybir.InstNoOp(
                                name=f"WH-{n}", engine=inst.engine, ins=[], outs=[],
                                sync_info=mybir.SyncInfo(on_wait=[w], on_update=[]),
                            )
                        )
                    si.on_wait = waits[-1:]
                new_insts.append(inst)
            block.instructions = new_insts
    return nc


def _get_nc():
    if "nc" not in _NC_CACHE:
        _NC_CACHE["nc"] = _split_multiwaits(_build_nc())
    return _NC_CACHE["nc"]


def _make_consts():
    import ml_dtypes

    bf16 = ml_dtypes.bfloat16
    ch = np.zeros((P, 1024), dtype=np.uint8)
    idb2 = np.zeros((P, H), dtype=bf16)
    idb2[0:H] = np.eye(H, dtype=bf16)
    idb2[H:P] = np.eye(H, dtype=bf16)
    ch[:, 0:128] = idb2.view(np.uint8)
    ch[:, 128:384] = np.eye(P, dtype=bf16).view(np.uint8)
    tri = np.triu(np.ones((P, P), dtype=np.float32)).astype(bf16)
    ch[:, 384:640] = tri.view(np.uint8)
    id65 = np.eye(H + 1, dtype=np.float32)
    ch[0 : H + 1, 640:900] = id65.view(np.uint8)
    return ch


def _make_in_maps(x, Wk, Wq, Wv):
    import ml_dtypes

    bf16 = ml_dtypes.bfloat16
    scale = 1.0 / np.sqrt(np.float32(C))
    w1 = np.concatenate([Wv, Wq], axis=0).T           # [C, 128] -> vT+qT
    w2 = np.concatenate([Wk * scale, Wk * scale], axis=0).T  # [C, 128] -> kT
    wh = np.ascontiguousarray(
        np.stack([w1, w2]).astype(bf16)
        .reshape(2, CB, P, P).transpose(2, 0, 1, 3)
    )
    ch = _make_consts()
    in_maps = []
    for b in range(B):
        xt = x[b].T.astype(bf16)  # [C, T]
        xhb = np.ascontiguousarray(
            xt.reshape(CB, P, TJ, 512).transpose(2, 1, 0, 3)
        )
        in_maps.append({"xh": xhb, "wh": wh, "ch": ch})
    return in_maps


def run(x, Wk, Wq, Wv, trace=False):
    from concourse.bass_utils import run_bass_kernel_spmd

    nc = _get_nc()
    in_maps = _make_in_maps(x, Wk, Wq, Wv)
    res = run_bass_kernel_spmd(nc, in_maps, core_ids=list(range(N_CORES)), trace=trace)
    out = np.stack([np.asarray(res.results[b]["out"]) for b in range(B)], axis=0)
    return out.astype(np.float32), res


def kernel(x, Wk, Wq, Wv):
    out, _ = run(x, Wk, Wq, Wv, trace=False)
    return out
